# revision 14
# baseline (speedup 1.0000x reference)
"""2-layer GAT (N=50000, E=800000, heads=1, 128->64->64) on 8 TRN2 NeuronCores.

Sharding: edges partitioned by dst range (core k owns dst in [k*6250, (k+1)*6250)),
so all segment reductions are core-local. The only collectives are two AllGathers
(one per layer) of the per-node table T = [h | alpha_src | alpha_dst | pad] that
every core gathers rows from.

Per core, per layer:
  - build table shard rows [6250, 128] = x_shard @ W.T (+ alpha columns), AllGather
    into T[1:50001] of a [50002, 128] f32 table (row 0 / row 50001 = pad sentinels
    with alpha_src = -1e4 so padded edge slots get weight exp(0.2*(-1e4)) == 0).
  - per window of 128 dst nodes: gather h[src] rows via two dma_gather calls
    (int16 indices limit 32767 -> region A = table rows [0,32767), region B =
    [32767, 50002)), compute w = exp(max(e, 0.2e)), e = alpha_s[src]+alpha_d[dst],
    and segment-sum msg = [w*h | w] into the window's PSUM bank via a one-hot
    matmul (Q[e,s] = (dst_local[e]==s)).
  - alpha_d[dst] per edge comes from a second small matmul: QT = (s==dst[e]) built
    from a ones-matmul partition-broadcast of dst_local, times the window's
    alpha_d column.
  - window epilogue: out = num/den (+bias, relu for layer 1).
Final output rows are per-core disjoint; host concatenates.
"""

import math
import numpy as np

import concourse.bass as bass
import concourse.bacc as bacc
import concourse.mybir as mybir
import concourse.tile as tile
from concourse.bass_utils import run_bass_kernel_spmd

N = 50000
E = 800000
C = 8
NPC = N // C            # 6250 nodes per core
WPC = math.ceil(NPC / 128)  # 49 windows per core (last window = 106 nodes)
P = 128
F_IN = 128
F_OUT = 64
TW = 128                # table row width (f32) -> 512B rows for dma_gather
COL_AS = 64             # alpha_src column in table row
COL_AD = 65             # alpha_dst column
NROWS = N + 2           # table rows: [sentinelA, nodes 0..N-1, sentinelB]
REG = 32767             # region A = rows [0, REG), region B = rows [REG, NROWS)
SENT_A_IDX = 0
SENT_B_IDX = NROWS - 1 - REG  # 17234
NEG_BIG = -1.0e4


# ---------------------------------------------------------------- host prep

def _wrap16(idx_i16: np.ndarray) -> np.ndarray:
    """[n] -> [16, n/16] wrapped (i -> [i%16, i//16]); n must be %16."""
    n = idx_i16.shape[0]
    a = np.empty((16, n // 16), np.int16)
    a[np.arange(n) % 16, np.arange(n) // 16] = idx_i16
    return a


def _prep_edges(edge_index: np.ndarray):
    """Partition edges (plus self-loops) into (core, window, region) groups.

    Returns per-core staged arrays + the global per-window tile counts TA, TB
    and the super-batch layout.
    """
    src = np.concatenate([edge_index[0], np.arange(N, dtype=np.int64)]).astype(np.int64)
    dst = np.concatenate([edge_index[1], np.arange(N, dtype=np.int64)]).astype(np.int64)
    core = dst // NPC
    rem = dst % NPC
    win = rem // 128
    dloc = rem % 128
    trow = src + 1                       # table row of src
    in_a = trow < REG

    # group edges by (core, window, region) via a single sort
    # key = core * (WPC*2) + win*2 + (0 if A else 1)
    key = core * (WPC * 2) + win * 2 + (~in_a).astype(np.int64)
    order = np.argsort(key, kind="stable")
    key_s = key[order]
    trow_s = trow[order]
    dloc_s = dloc[order]
    # counts per group
    counts = np.bincount(key_s, minlength=C * WPC * 2).reshape(C, WPC, 2)
    TA = np.ceil(counts[:, :, 0].max(axis=0) / 128).astype(np.int64)  # [WPC]
    TB = np.ceil(counts[:, :, 1].max(axis=0) / 128).astype(np.int64)
    TA = np.maximum(TA, 1)
    TB = np.maximum(TB, 1)

    # super-batches: pairs of windows
    sbs = [(w, w + 1) if w + 1 < WPC else (w,) for w in range(0, WPC, 2)]

    NT = int((TA + TB).sum())           # total tiles per core per layer
    starts = np.zeros(C * WPC * 2 + 1, np.int64)
    np.cumsum(counts.reshape(-1), out=starts[1:])

    per_core = []
    for k in range(C):
        idxA_cols = []
        idxB_cols = []
        dcol = np.zeros((P, NT), np.float32)
        drow = np.zeros((1, NT * P), np.float32)
        g = 0
        for sb in sbs:
            for region in (0, 1):
                T_arr = TA if region == 0 else TB
                pad_idx = SENT_A_IDX if region == 0 else SENT_B_IDX
                chunks = []
                for w in sb:
                    gi = k * (WPC * 2) + w * 2 + region
                    n_e = counts[k, w, region]
                    s0 = starts[gi]
                    rows = trow_s[s0 : s0 + n_e]
                    if region == 0:
                        ridx = rows.astype(np.int16)
                    else:
                        ridx = (rows - REG).astype(np.int16)
                    nslot = int(T_arr[w]) * 128
                    idx = np.full(nslot, pad_idx, np.int16)
                    idx[:n_e] = ridx
                    dl = np.zeros(nslot, np.float32)
                    dl[:n_e] = dloc_s[s0 : s0 + n_e].astype(np.float32)
                    chunks.append((idx, dl, int(T_arr[w])))
                idx_cat = np.concatenate([c[0] for c in chunks])
                (idxA_cols if region == 0 else idxB_cols).append(
                    np.tile(_wrap16(idx_cat), (8, 1))
                )
                # dst_local staging for this region's tiles
                for _, dl, ntile in chunks:
                    for t in range(ntile):
                        seg = dl[t * 128 : (t + 1) * 128]
                        dcol[:, g] = seg
                        drow[0, g * P : (g + 1) * P] = seg
                        g += 1
        assert g == NT
        per_core.append(
            dict(
                idxA=np.concatenate(idxA_cols, axis=1),
                idxB=np.concatenate(idxB_cols, axis=1),
                dcol=dcol,
                drow=drow,
            )
        )
    return per_core, TA, TB, sbs, NT


# ---------------------------------------------------------- device program

def _build_program(TA, TB, sbs, NT, idxA_cols_total, idxB_cols_total):
    nc = bacc.Bacc(None, target_bir_lowering=False, debug=False)
    dt = mybir.dt

    x_in = nc.declare_dram_parameter("x_shard", [WPC * 128, F_IN], dt.float32, isOutput=False)
    W1T_in = nc.declare_dram_parameter("W1T", [F_IN, F_OUT], dt.float32, isOutput=False)
    W1_in = nc.declare_dram_parameter("W1", [F_OUT, F_IN], dt.float32, isOutput=False)
    A1_in = nc.declare_dram_parameter("A1", [F_OUT, 2], dt.float32, isOutput=False)
    b1_in = nc.declare_dram_parameter("b1", [1, F_OUT], dt.float32, isOutput=False)
    W2T_in = nc.declare_dram_parameter("W2T", [F_OUT, F_OUT], dt.float32, isOutput=False)
    W2_in = nc.declare_dram_parameter("W2", [F_OUT, F_OUT], dt.float32, isOutput=False)
    A2_in = nc.declare_dram_parameter("A2", [F_OUT, 2], dt.float32, isOutput=False)
    b2_in = nc.declare_dram_parameter("b2", [1, F_OUT], dt.float32, isOutput=False)
    idxA_in = nc.declare_dram_parameter("idxA", [P, idxA_cols_total], dt.int16, isOutput=False)
    idxB_in = nc.declare_dram_parameter("idxB", [P, idxB_cols_total], dt.int16, isOutput=False)
    dcol_in = nc.declare_dram_parameter("dcol", [P, NT], dt.float32, isOutput=False)
    drow_in = nc.declare_dram_parameter("drow", [1, NT * P], dt.float32, isOutput=False)
    out_p = nc.declare_dram_parameter("out", [NPC, F_OUT], dt.float32, isOutput=True)

    ag1_in = nc.dram_tensor("ag1_in", [NPC, TW], dt.float32)
    ag2_in = nc.dram_tensor("ag2_in", [NPC, TW], dt.float32)
    T1 = nc.dram_tensor("T1", [NROWS, TW], dt.float32, addr_space="Shared")
    T2 = nc.dram_tensor("T2", [NROWS, TW], dt.float32, addr_space="Shared")

    iota_row_np = np.broadcast_to(np.arange(P, dtype=np.float32), (P, P)).copy()
    iota_row_d = nc.inline_tensor(iota_row_np, "iota_row")
    iotaP_np = np.arange(P, dtype=np.float32)[:, None].copy()
    iotaP_d = nc.inline_tensor(iotaP_np, "iotaP")
    ident_d = nc.inline_tensor(np.eye(P, dtype=np.float32), "ident")

    rg = [list(range(C))]
    T_sb_max = max(
        int(sum(TA[w] + TB[w] for w in sb)) for sb in sbs
    )

    with tile.TileContext(nc) as tc:
        with (
            tc.tile_pool(name="cons", bufs=1) as cons,
            tc.tile_pool(name="persist", bufs=1) as persist,
            tc.tile_pool(name="sb", bufs=2) as sbp,
        ):
            # ---- constants
            iota_row = cons.tile([P, P], dt.float32)
            nc.sync.dma_start(out=iota_row[:], in_=iota_row_d[:, :])
            iotaP = cons.tile([P, 1], dt.float32)
            nc.sync.dma_start(out=iotaP[:], in_=iotaP_d[:, :])
            ident = cons.tile([P, P], dt.float32)
            nc.sync.dma_start(out=ident[:], in_=ident_d[:, :])
            ones_row = cons.tile([1, P], dt.float32)
            nc.vector.memset(ones_row[:], 1.0)

            def load_const(dram, shape, nm):
                t = cons.tile(shape, dt.float32, tag=nm, name=nm)
                nc.sync.dma_start(out=t[:], in_=dram[:, :])
                return t

            W1T = load_const(W1T_in, [F_IN, F_OUT], "c_W1T")
            W1 = load_const(W1_in, [F_OUT, F_IN], "c_W1")
            A1 = load_const(A1_in, [F_OUT, 2], "c_A1")
            b1r = load_const(b1_in, [1, F_OUT], "c_b1")
            W2T = load_const(W2T_in, [F_OUT, F_OUT], "c_W2T")
            W2 = load_const(W2_in, [F_OUT, F_OUT], "c_W2")
            A2 = load_const(A2_in, [F_OUT, 2], "c_A2")
            b2r = load_const(b2_in, [1, F_OUT], "c_b2")

            # wa[i, :] = sum_o W[o, i] * [a_src, a_dst][o]
            def mk_wa(ps, W_sb, A_sb, k_dim):
                p = ps.tile([F_IN, 2], dt.float32, tag="trp", name=f"wa_ps{k_dim}")
                nc.tensor.matmul(out=p[0:k_dim, :], lhsT=W_sb[:], rhs=A_sb[:], start=True, stop=True)
                s = cons.tile([k_dim, 2], dt.float32, tag=f"wa{k_dim}")
                nc.vector.tensor_copy(out=s[:], in_=p[0:k_dim, :])
                return s

            def mk_bcast(ps, row_sb):
                p = ps.tile([P, F_OUT], dt.float32, tag="hp", name=f"b_ps_{row_sb.tensor.name}")
                nc.tensor.matmul(out=p[:], lhsT=ones_row[:], rhs=row_sb[:], start=True, stop=True)
                s = cons.tile([P, F_OUT], dt.float32, tag=f"bb{row_sb.tensor.name}")
                nc.vector.tensor_copy(out=s[:], in_=p[:])
                return s

            r1_sb = persist.tile([P, WPC, F_OUT], dt.float32)

            sent = cons.tile([1, TW], dt.float32)
            nc.vector.memset(sent[:], 0.0)
            nc.vector.memset(sent[0:1, COL_AS : COL_AS + 1], NEG_BIG)

            # ---------------- table build for one layer
            def build_table(layer, ps, wa):
                WT = W1T if layer == 1 else W2T
                ag_in = ag1_in if layer == 1 else ag2_in
                T = T1 if layer == 1 else T2
                k_dim = F_IN if layer == 1 else F_OUT
                for t in range(WPC):
                    if layer == 1:
                        xt = sbp.tile([P, F_IN], dt.float32, tag="xt")
                        nc.sync.dma_start(out=xt[:], in_=x_in[t * 128 : (t + 1) * 128, :])
                        src_ap = xt[:]
                    else:
                        src_ap = r1_sb[:, t, :]
                    trp = ps.tile([P, P], dt.float32, tag="trp")
                    nc.tensor.transpose(out=trp[0:k_dim, :], in_=src_ap, identity=ident[:])
                    xT = sbp.tile([k_dim, P], dt.float32, tag="xT")
                    nc.vector.tensor_copy(out=xT[:], in_=trp[0:k_dim, :])
                    hp = ps.tile([P, F_OUT], dt.float32, tag="hp")
                    nc.tensor.matmul(out=hp[:], lhsT=xT[:], rhs=WT[:], start=True, stop=True)
                    ap_ = ps.tile([P, 2], dt.float32, tag="ap_")
                    nc.tensor.matmul(out=ap_[:], lhsT=xT[:], rhs=wa[:], start=True, stop=True)
                    row = sbp.tile([P, F_OUT + 2], dt.float32, tag="row")
                    nc.vector.tensor_copy(out=row[:, 0:F_OUT], in_=hp[:])
                    nc.vector.tensor_copy(out=row[:, F_OUT : F_OUT + 2], in_=ap_[:])
                    nrows = min(128, NPC - t * 128)
                    nc.sync.dma_start(
                        out=ag_in[t * 128 : t * 128 + nrows, 0 : F_OUT + 2],
                        in_=row[0:nrows, :],
                    )
                nc.gpsimd.collective_compute(
                    "AllGather",
                    mybir.AluOpType.bypass,
                    replica_groups=rg,
                    ins=[ag_in.ap().opt()],
                    outs=[T[1 : N + 1, :].opt()],
                )
                nc.sync.dma_start(out=T[0:1, :], in_=sent[:])
                nc.sync.dma_start(out=T[NROWS - 1 : NROWS, :], in_=sent[:])

            # ---------------- edge phase for one layer
            def edge_phase(layer, ps, winps, bb):
                T = T1 if layer == 1 else T2
                colA = 0
                colB = 0
                g0 = 0
                for sb in sbs:
                    TA_sb = int(sum(TA[w] for w in sb))
                    TB_sb = int(sum(TB[w] for w in sb))
                    T_sb = TA_sb + TB_sb
                    ia = sbp.tile([P, TA_sb * 8], dt.int16, tag="ia")
                    nc.sync.dma_start(out=ia[:], in_=idxA_in[:, colA : colA + TA_sb * 8])
                    ib = sbp.tile([P, TB_sb * 8], dt.int16, tag="ib")
                    nc.sync.dma_start(out=ib[:], in_=idxB_in[:, colB : colB + TB_sb * 8])
                    dcol = sbp.tile([P, T_sb], dt.float32, tag="dcol")
                    nc.sync.dma_start(out=dcol[:], in_=dcol_in[:, g0 : g0 + T_sb])
                    drow = sbp.tile([1, T_sb * P], dt.float32, tag="drow")
                    nc.sync.dma_start(out=drow[:], in_=drow_in[:, g0 * P : (g0 + T_sb) * P])

                    gath = sbp.tile([P, T_sb, TW], dt.float32, tag="gath")
                    nc.gpsimd.dma_gather(
                        out_ap=gath[:, 0:TA_sb, :],
                        in_ap=T[0:REG, :],
                        idxs_ap=ia[:],
                        num_idxs=TA_sb * 128,
                        num_idxs_reg=TA_sb * 128,
                        elem_size=TW,
                        single_packet=False,
                    )
                    nc.gpsimd.dma_gather(
                        out_ap=gath[:, TA_sb:T_sb, :],
                        in_ap=T[REG:NROWS, :],
                        idxs_ap=ib[:],
                        num_idxs=TB_sb * 128,
                        num_idxs_reg=TB_sb * 128,
                        elem_size=TW,
                        single_packet=False,
                    )

                    # alpha_d for this super-batch's windows, read from this
                    # core's OWN shard bounce (core-local rows -> same program
                    # on every core).
                    ag_in = ag1_in if layer == 1 else ag2_in
                    ad_wins = {}
                    for w in sb:
                        nw = min(128, NPC - w * 128)
                        adw = sbp.tile([P, 1], dt.float32, tag="adw")
                        if nw < 128:
                            nc.vector.memset(adw[:], 0.0)
                        nc.sync.dma_start(
                            out=adw[0:nw, :],
                            in_=ag_in[w * 128 : w * 128 + nw, COL_AD : COL_AD + 1],
                        )
                        ad_wins[w] = adw

                    # per-tile PE chain: alpha_d per edge
                    adps = ps.tile([P, T_sb], dt.float32, tag="adps")
                    for t in range(T_sb):
                        w = _tile_window(sb, TA, TB, t)
                        bcp = ps.tile([P, P], dt.float32, tag="bcp")
                        nc.tensor.matmul(
                            out=bcp[:],
                            lhsT=ones_row[:],
                            rhs=drow[0:1, t * P : (t + 1) * P],
                            start=True,
                            stop=True,
                        )
                        qt = sbp.tile([P, P], dt.float32, tag="qt")
                        nc.vector.tensor_scalar(
                            out=qt[:], in0=bcp[:], scalar1=iotaP[:, 0:1], scalar2=None,
                            op0=mybir.AluOpType.is_equal,
                        )
                        nc.tensor.matmul(
                            out=adps[:, t : t + 1],
                            lhsT=qt[:],
                            rhs=ad_wins[w][:],
                            start=True,
                            stop=True,
                        )

                    # e = alpha_s + alpha_d ; w = exp(max(e, 0.2e))
                    et = sbp.tile([P, T_sb, 1], dt.float32, tag="et")
                    nc.vector.tensor_tensor(
                        out=et[:],
                        in0=gath[:, :, COL_AS : COL_AS + 1],
                        in1=adps[:, :, None],
                        op=mybir.AluOpType.add,
                    )
                    e2 = sbp.tile([P, T_sb, 1], dt.float32, tag="e2")
                    nc.vector.tensor_scalar(
                        out=e2[:], in0=et[:], scalar1=0.2, scalar2=None,
                        op0=mybir.AluOpType.mult,
                    )
                    nc.vector.tensor_tensor(
                        out=e2[:], in0=et[:], in1=e2[:], op=mybir.AluOpType.max,
                    )
                    wt = sbp.tile([P, T_sb, 1], dt.float32, tag="wt")
                    nc.scalar.activation(
                        out=wt[:], in_=e2[:], func=mybir.ActivationFunctionType.Exp,
                    )

                    msg = sbp.tile([P, T_sb, F_OUT + 1], dt.float32, tag="msg")
                    nc.vector.tensor_tensor(
                        out=msg[:, :, 0:F_OUT],
                        in0=gath[:, :, 0:F_OUT],
                        in1=wt[:].to_broadcast([P, T_sb, F_OUT]),
                        op=mybir.AluOpType.mult,
                    )
                    nc.vector.tensor_copy(out=msg[:, :, F_OUT : F_OUT + 1], in_=wt[:])

                    # segment-sum matmuls
                    win_ps = {
                        w: winps.tile([P, F_OUT + 1], dt.float32, tag="win", name=f"win_l{layer}_w{w}")
                        for w in sb
                    }
                    first_seen = {w: True for w in sb}
                    remaining = {w: int(TA[w] + TB[w]) for w in sb}
                    for t in range(T_sb):
                        w = _tile_window(sb, TA, TB, t)
                        q = sbp.tile([P, P], dt.float32, tag="q")
                        nc.vector.tensor_scalar(
                            out=q[:], in0=iota_row[:], scalar1=dcol[:, t : t + 1], scalar2=None,
                            op0=mybir.AluOpType.is_equal,
                        )
                        nc.tensor.matmul(
                            out=win_ps[w][:],
                            lhsT=q[:],
                            rhs=msg[:, t, :],
                            start=first_seen[w],
                            stop=(remaining[w] == 1),
                        )
                        first_seen[w] = False
                        remaining[w] -= 1

                    # window epilogues
                    for w in sb:
                        pw = win_ps[w]
                        rec = sbp.tile([P, 1], dt.float32, tag="rec")
                        nc.vector.reciprocal(out=rec[:], in_=pw[:, F_OUT : F_OUT + 1])
                        rt = sbp.tile([P, F_OUT], dt.float32, tag="rt")
                        nc.vector.tensor_scalar(
                            out=rt[:], in0=pw[:, 0:F_OUT], scalar1=rec[:, 0:1], scalar2=None,
                            op0=mybir.AluOpType.mult,
                        )
                        nc.vector.tensor_tensor(
                            out=rt[:], in0=rt[:], in1=bb[:], op=mybir.AluOpType.add,
                        )
                        if layer == 1:
                            nc.scalar.activation(
                                out=r1_sb[:, w, :], in_=rt[:],
                                func=mybir.ActivationFunctionType.Relu,
                            )
                        else:
                            nw = min(128, NPC - w * 128)
                            nc.sync.dma_start(
                                out=out_p[w * 128 : w * 128 + nw, :], in_=rt[0:nw, :],
                            )

                    colA += TA_sb * 8
                    colB += TB_sb * 8
                    g0 += T_sb

            with tc.tile_pool(name="psA1", bufs=2, space="PSUM") as psA:
                w1a = mk_wa(psA, W1, A1, F_IN)
                w2a = mk_wa(psA, W2, A2, F_OUT)
                b1b = mk_bcast(psA, b1r)
                b2b = mk_bcast(psA, b2r)
                build_table(1, psA, w1a)
            with tc.tile_pool(name="psE1", bufs=2, space="PSUM") as psE:
                with tc.tile_pool(name="winps1", bufs=2, space="PSUM") as winps:
                    edge_phase(1, psE, winps, b1b)
            with tc.tile_pool(name="psA2", bufs=2, space="PSUM") as psA:
                build_table(2, psA, w2a)
            with tc.tile_pool(name="psE2", bufs=2, space="PSUM") as psE:
                with tc.tile_pool(name="winps2", bufs=2, space="PSUM") as winps:
                    edge_phase(2, psE, winps, b2b)

    nc.compile()
    return nc


def _tile_window(sb, TA, TB, t):
    """Map tile index t within a super-batch to its window id."""
    # layout: [A(w) for w in sb] then [B(w) for w in sb]
    off = 0
    for w in sb:
        if t < off + int(TA[w]):
            return w
        off += int(TA[w])
    for w in sb:
        if t < off + int(TB[w]):
            return w
        off += int(TB[w])
    raise AssertionError("tile out of range")


_CACHE = {}


def kernel(**inputs) -> np.ndarray:
    x = np.asarray(inputs["x"], np.float32)
    edge_index = np.asarray(inputs["edge_index"])
    W1 = np.asarray(inputs["W1"], np.float32)
    a1s = np.asarray(inputs["a1_src"], np.float32)
    a1d = np.asarray(inputs["a1_dst"], np.float32)
    b1 = np.asarray(inputs["b1"], np.float32)
    W2 = np.asarray(inputs["W2"], np.float32)
    a2s = np.asarray(inputs["a2_src"], np.float32)
    a2d = np.asarray(inputs["a2_dst"], np.float32)
    b2 = np.asarray(inputs["b2"], np.float32)

    per_core, TA, TB, sbs, NT = _prep_edges(edge_index)

    nc = _build_program(
        TA, TB, sbs, NT,
        per_core[0]["idxA"].shape[1],
        per_core[0]["idxB"].shape[1],
    )

    in_maps = []
    for k in range(C):
        xs = np.zeros((WPC * 128, F_IN), np.float32)
        xs[0:NPC] = x[k * NPC : (k + 1) * NPC]
        in_maps.append(
            {
                "x_shard": xs,
                "W1T": np.ascontiguousarray(W1.T),
                "W1": np.ascontiguousarray(W1),
                "A1": np.ascontiguousarray(np.stack([a1s, a1d], axis=1)),
                "b1": b1[None, :].copy(),
                "W2T": np.ascontiguousarray(W2.T),
                "W2": np.ascontiguousarray(W2),
                "A2": np.ascontiguousarray(np.stack([a2s, a2d], axis=1)),
                "b2": b2[None, :].copy(),
                "idxA": per_core[k]["idxA"],
                "idxB": per_core[k]["idxB"],
                "dcol": per_core[k]["dcol"],
                "drow": per_core[k]["drow"],
            }
        )

    res = run_bass_kernel_spmd(nc, in_maps, core_ids=list(range(C)))
    out = np.concatenate([res.results[k]["out"] for k in range(C)], axis=0)
    return out.astype(np.float32)


# revision 19
# speedup vs baseline: 1.3594x; 1.3594x over previous
"""2-layer GAT (N=50000, E=800000, heads=1, 128->64->64) on 8 TRN2 NeuronCores.

Sharding: edges partitioned by dst range (core k owns dst in [k*6250, (k+1)*6250)),
so all segment reductions are core-local. The only collectives are two AllGathers
(one per layer) of the per-node table T = [h | alpha_src | alpha_dst | pad] that
every core gathers rows from.

Per core, per layer:
  - build table shard rows [6250, 128] = x_shard @ W.T (+ alpha columns), AllGather
    into T[1:50001] of a [50002, 128] f32 table (row 0 / row 50001 = pad sentinels
    with alpha_src = -1e4 so padded edge slots get weight exp(0.2*(-1e4)) == 0).
  - per window of 128 dst nodes: gather h[src] rows via two dma_gather calls
    (int16 indices limit 32767 -> region A = table rows [0,32767), region B =
    [32767, 50002)), compute w = exp(max(e, 0.2e)), e = alpha_s[src]+alpha_d[dst],
    and segment-sum msg = [w*h | w] into the window's PSUM bank via a one-hot
    matmul (Q[e,s] = (dst_local[e]==s)).
  - alpha_d[dst] per edge comes from a second small matmul: QT = (s==dst[e]) built
    from a ones-matmul partition-broadcast of dst_local, times the window's
    alpha_d column.
  - window epilogue: out = num/den (+bias, relu for layer 1).
Final output rows are per-core disjoint; host concatenates.
"""

import math
import os
import numpy as np

ABLATE = set(os.environ.get("GAT_ABLATE", "").split(",")) - {""}

import concourse.bass as bass
import concourse.bacc as bacc
import concourse.mybir as mybir
import concourse.tile as tile
from concourse.bass_utils import run_bass_kernel_spmd

N = 50000
E = 800000
C = 8
NPC = N // C            # 6250 nodes per core
WPC = math.ceil(NPC / 128)  # 49 windows per core (last window = 106 nodes)
P = 128
F_IN = 128
F_OUT = 64
TW = 128                # table row width (f32) -> 512B rows for dma_gather
COL_AS = 64             # alpha_src column in table row
COL_AD = 65             # alpha_dst column
NROWS = N + 2           # table rows: [sentinelA, nodes 0..N-1, sentinelB]
REG = 32767             # region A = rows [0, REG), region B = rows [REG, NROWS)
SENT_A_IDX = 0
SENT_B_IDX = NROWS - 1 - REG  # 17234
NEG_BIG = -1.0e4


# ---------------------------------------------------------------- host prep

def _wrap16(idx_i16: np.ndarray) -> np.ndarray:
    """[n] -> [16, n/16] wrapped (i -> [i%16, i//16]); n must be %16."""
    n = idx_i16.shape[0]
    a = np.empty((16, n // 16), np.int16)
    a[np.arange(n) % 16, np.arange(n) // 16] = idx_i16
    return a


def _prep_edges(edge_index: np.ndarray):
    """Partition edges (plus self-loops) into (core, window, region) groups.

    Returns per-core staged arrays + the global per-window tile counts TA, TB
    and the super-batch layout.
    """
    src = np.concatenate([edge_index[0], np.arange(N, dtype=np.int64)]).astype(np.int64)
    dst = np.concatenate([edge_index[1], np.arange(N, dtype=np.int64)]).astype(np.int64)
    core = dst // NPC
    rem = dst % NPC
    win = rem // 128
    dloc = rem % 128
    trow = src + 1                       # table row of src
    in_a = trow < REG

    # group edges by (core, window, region) via a single sort
    # key = core * (WPC*2) + win*2 + (0 if A else 1)
    key = core * (WPC * 2) + win * 2 + (~in_a).astype(np.int64)
    order = np.argsort(key, kind="stable")
    key_s = key[order]
    trow_s = trow[order]
    dloc_s = dloc[order]
    # counts per group
    counts = np.bincount(key_s, minlength=C * WPC * 2).reshape(C, WPC, 2)
    TA = np.ceil(counts[:, :, 0].max(axis=0) / 128).astype(np.int64)  # [WPC]
    TB = np.ceil(counts[:, :, 1].max(axis=0) / 128).astype(np.int64)
    TA = np.maximum(TA, 1)
    TB = np.maximum(TB, 1)

    # super-batches: pairs of windows
    sbs = [(w, w + 1) if w + 1 < WPC else (w,) for w in range(0, WPC, 2)]

    NT = int((TA + TB).sum())           # total tiles per core per layer
    starts = np.zeros(C * WPC * 2 + 1, np.int64)
    np.cumsum(counts.reshape(-1), out=starts[1:])

    per_core = []
    for k in range(C):
        idxA_cols = []
        idxB_cols = []
        dcol = np.zeros((P, NT), np.float32)
        drow = np.zeros((1, NT * P), np.float32)
        g = 0
        for sb in sbs:
            for region in (0, 1):
                T_arr = TA if region == 0 else TB
                pad_idx = SENT_A_IDX if region == 0 else SENT_B_IDX
                chunks = []
                for w in sb:
                    gi = k * (WPC * 2) + w * 2 + region
                    n_e = counts[k, w, region]
                    s0 = starts[gi]
                    rows = trow_s[s0 : s0 + n_e]
                    if region == 0:
                        ridx = rows.astype(np.int16)
                    else:
                        ridx = (rows - REG).astype(np.int16)
                    nslot = int(T_arr[w]) * 128
                    idx = np.full(nslot, pad_idx, np.int16)
                    idx[:n_e] = ridx
                    dl = np.zeros(nslot, np.float32)
                    dl[:n_e] = dloc_s[s0 : s0 + n_e].astype(np.float32)
                    chunks.append((idx, dl, int(T_arr[w])))
                idx_cat = np.concatenate([c[0] for c in chunks])
                (idxA_cols if region == 0 else idxB_cols).append(
                    np.tile(_wrap16(idx_cat), (8, 1))
                )
                # dst_local staging for this region's tiles
                for _, dl, ntile in chunks:
                    for t in range(ntile):
                        seg = dl[t * 128 : (t + 1) * 128]
                        dcol[:, g] = seg
                        drow[0, g * P : (g + 1) * P] = seg
                        g += 1
        assert g == NT
        per_core.append(
            dict(
                idxA=np.concatenate(idxA_cols, axis=1),
                idxB=np.concatenate(idxB_cols, axis=1),
                dcol=dcol,
                drow=drow,
            )
        )
    return per_core, TA, TB, sbs, NT


# ---------------------------------------------------------- device program

def _build_program(TA, TB, sbs, NT, idxA_cols_total, idxB_cols_total):
    nc = bacc.Bacc(None, target_bir_lowering=False, debug=False)
    dt = mybir.dt

    x_in = nc.declare_dram_parameter("x_shard", [WPC * 128, F_IN], dt.float32, isOutput=False)
    W1T_in = nc.declare_dram_parameter("W1T", [F_IN, F_OUT], dt.float32, isOutput=False)
    W1_in = nc.declare_dram_parameter("W1", [F_OUT, F_IN], dt.float32, isOutput=False)
    A1_in = nc.declare_dram_parameter("A1", [F_OUT, 2], dt.float32, isOutput=False)
    b1_in = nc.declare_dram_parameter("b1", [1, F_OUT], dt.float32, isOutput=False)
    W2T_in = nc.declare_dram_parameter("W2T", [F_OUT, F_OUT], dt.float32, isOutput=False)
    W2_in = nc.declare_dram_parameter("W2", [F_OUT, F_OUT], dt.float32, isOutput=False)
    A2_in = nc.declare_dram_parameter("A2", [F_OUT, 2], dt.float32, isOutput=False)
    b2_in = nc.declare_dram_parameter("b2", [1, F_OUT], dt.float32, isOutput=False)
    idxA_in = nc.declare_dram_parameter("idxA", [P, idxA_cols_total], dt.int16, isOutput=False)
    idxB_in = nc.declare_dram_parameter("idxB", [P, idxB_cols_total], dt.int16, isOutput=False)
    dcol_in = nc.declare_dram_parameter("dcol", [P, NT], dt.float32, isOutput=False)
    drow_in = nc.declare_dram_parameter("drow", [1, NT * P], dt.float32, isOutput=False)
    out_p = nc.declare_dram_parameter("out", [NPC, F_OUT], dt.float32, isOutput=True)

    ag1_in = nc.dram_tensor("ag1_in", [NPC, TW], dt.float32)
    ag2_in = nc.dram_tensor("ag2_in", [NPC, TW], dt.float32)
    T1 = nc.dram_tensor("T1", [NROWS, TW], dt.float32, addr_space="Shared")
    T2 = nc.dram_tensor("T2", [NROWS, TW], dt.float32, addr_space="Shared")

    iota_row_np = np.broadcast_to(np.arange(P, dtype=np.float32), (P, P)).copy()
    iota_row_d = nc.inline_tensor(iota_row_np, "iota_row")
    iotaP_np = np.arange(P, dtype=np.float32)[:, None].copy()
    iotaP_d = nc.inline_tensor(iotaP_np, "iotaP")
    ident_d = nc.inline_tensor(np.eye(P, dtype=np.float32), "ident")

    rg = [list(range(C))]
    T_sb_max = max(
        int(sum(TA[w] + TB[w] for w in sb)) for sb in sbs
    )

    with tile.TileContext(nc) as tc:
        with (
            tc.tile_pool(name="cons", bufs=1) as cons,
            tc.tile_pool(name="persist", bufs=1) as persist,
            tc.tile_pool(name="sb", bufs=2) as sbp,
        ):
            # ---- constants
            iota_row = cons.tile([P, P], dt.float32)
            nc.sync.dma_start(out=iota_row[:], in_=iota_row_d[:, :])
            iotaP = cons.tile([P, 1], dt.float32)
            nc.sync.dma_start(out=iotaP[:], in_=iotaP_d[:, :])
            ident = cons.tile([P, P], dt.float32)
            nc.sync.dma_start(out=ident[:], in_=ident_d[:, :])
            ones_row = cons.tile([1, P], dt.float32)
            nc.vector.memset(ones_row[:], 1.0)

            def load_const(dram, shape, nm):
                t = cons.tile(shape, dt.float32, tag=nm, name=nm)
                nc.sync.dma_start(out=t[:], in_=dram[:, :])
                return t

            W1T = load_const(W1T_in, [F_IN, F_OUT], "c_W1T")
            W1 = load_const(W1_in, [F_OUT, F_IN], "c_W1")
            A1 = load_const(A1_in, [F_OUT, 2], "c_A1")
            b1r = load_const(b1_in, [1, F_OUT], "c_b1")
            W2T = load_const(W2T_in, [F_OUT, F_OUT], "c_W2T")
            W2 = load_const(W2_in, [F_OUT, F_OUT], "c_W2")
            A2 = load_const(A2_in, [F_OUT, 2], "c_A2")
            b2r = load_const(b2_in, [1, F_OUT], "c_b2")

            # wa[i, :] = sum_o W[o, i] * [a_src, a_dst][o]
            def mk_wa(ps, W_sb, A_sb, k_dim):
                p = ps.tile([F_IN, 2], dt.float32, tag="trp", name=f"wa_ps{k_dim}")
                nc.tensor.matmul(out=p[0:k_dim, :], lhsT=W_sb[:], rhs=A_sb[:], start=True, stop=True)
                s = cons.tile([k_dim, 2], dt.float32, tag=f"wa{k_dim}")
                nc.vector.tensor_copy(out=s[:], in_=p[0:k_dim, :])
                return s

            def mk_bcast(ps, row_sb):
                p = ps.tile([P, F_OUT], dt.float32, tag="hp", name=f"b_ps_{row_sb.tensor.name}")
                nc.tensor.matmul(out=p[:], lhsT=ones_row[:], rhs=row_sb[:], start=True, stop=True)
                s = cons.tile([P, F_OUT], dt.float32, tag=f"bb{row_sb.tensor.name}")
                nc.vector.tensor_copy(out=s[:], in_=p[:])
                return s

            r1_sb = persist.tile([P, WPC, F_OUT], dt.float32)

            sent = cons.tile([1, TW], dt.float32)
            nc.vector.memset(sent[:], 0.0)
            nc.vector.memset(sent[0:1, COL_AS : COL_AS + 1], NEG_BIG)

            # ---------------- table build for one layer
            def build_table(layer, ps, wa):
                WT = W1T if layer == 1 else W2T
                ag_in = ag1_in if layer == 1 else ag2_in
                T = T1 if layer == 1 else T2
                k_dim = F_IN if layer == 1 else F_OUT
                for t in range(WPC):
                    if layer == 1:
                        xt = sbp.tile([P, F_IN], dt.float32, tag="xt")
                        nc.sync.dma_start(out=xt[:], in_=x_in[t * 128 : (t + 1) * 128, :])
                        src_ap = xt[:]
                    else:
                        src_ap = r1_sb[:, t, :]
                    trp = ps.tile([P, P], dt.float32, tag="trp")
                    nc.tensor.transpose(out=trp[0:k_dim, :], in_=src_ap, identity=ident[:])
                    xT = sbp.tile([k_dim, P], dt.float32, tag="xT")
                    nc.vector.tensor_copy(out=xT[:], in_=trp[0:k_dim, :])
                    hp = ps.tile([P, F_OUT], dt.float32, tag="hp")
                    nc.tensor.matmul(out=hp[:], lhsT=xT[:], rhs=WT[:], start=True, stop=True)
                    ap_ = ps.tile([P, 2], dt.float32, tag="ap_")
                    nc.tensor.matmul(out=ap_[:], lhsT=xT[:], rhs=wa[:], start=True, stop=True)
                    row = sbp.tile([P, F_OUT + 2], dt.float32, tag="row")
                    nc.vector.tensor_copy(out=row[:, 0:F_OUT], in_=hp[:])
                    nc.vector.tensor_copy(out=row[:, F_OUT : F_OUT + 2], in_=ap_[:])
                    nrows = min(128, NPC - t * 128)
                    nc.sync.dma_start(
                        out=ag_in[t * 128 : t * 128 + nrows, 0 : F_OUT + 2],
                        in_=row[0:nrows, :],
                    )
                nc.gpsimd.collective_compute(
                    "AllGather",
                    mybir.AluOpType.bypass,
                    replica_groups=rg,
                    ins=[ag_in.ap().opt()],
                    outs=[T[1 : N + 1, :].opt()],
                )
                nc.sync.dma_start(out=T[0:1, :], in_=sent[:])
                nc.sync.dma_start(out=T[NROWS - 1 : NROWS, :], in_=sent[:])

            # ---------------- edge phase for one layer
            def edge_phase(layer, ps, winps, bb):
                T = T1 if layer == 1 else T2
                colA = 0
                colB = 0
                g0 = 0
                for sb in sbs:
                    TA_sb = int(sum(TA[w] for w in sb))
                    TB_sb = int(sum(TB[w] for w in sb))
                    T_sb = TA_sb + TB_sb
                    ia = sbp.tile([P, TA_sb * 8], dt.int16, tag="ia")
                    nc.sync.dma_start(out=ia[:], in_=idxA_in[:, colA : colA + TA_sb * 8])
                    ib = sbp.tile([P, TB_sb * 8], dt.int16, tag="ib")
                    nc.sync.dma_start(out=ib[:], in_=idxB_in[:, colB : colB + TB_sb * 8])
                    dcol = sbp.tile([P, T_sb], dt.float32, tag="dcol")
                    nc.sync.dma_start(out=dcol[:], in_=dcol_in[:, g0 : g0 + T_sb])
                    drow = sbp.tile([1, T_sb * P], dt.float32, tag="drow")
                    nc.sync.dma_start(out=drow[:], in_=drow_in[:, g0 * P : (g0 + T_sb) * P])

                    gath = sbp.tile([P, T_sb, TW], dt.float32, tag="gath")
                    if "no_gather" not in ABLATE:
                        nc.gpsimd.dma_gather(
                            out_ap=gath[:, 0:TA_sb, :],
                            in_ap=T[0:REG, :],
                            idxs_ap=ia[:],
                            num_idxs=TA_sb * 128,
                            num_idxs_reg=TA_sb * 128,
                            elem_size=TW,
                            single_packet=False,
                        )
                        nc.gpsimd.dma_gather(
                            out_ap=gath[:, TA_sb:T_sb, :],
                            in_ap=T[REG:NROWS, :],
                            idxs_ap=ib[:],
                            num_idxs=TB_sb * 128,
                            num_idxs_reg=TB_sb * 128,
                            elem_size=TW,
                            single_packet=False,
                        )
                    else:
                        nc.vector.memset(gath[:, 0:1, :], 0.5)

                    # alpha_d for this super-batch's windows, read from this
                    # core's OWN shard bounce (core-local rows -> same program
                    # on every core).
                    ag_in = ag1_in if layer == 1 else ag2_in
                    ad_wins = {}
                    for w in sb:
                        nw = min(128, NPC - w * 128)
                        adw = sbp.tile([P, 1], dt.float32, tag="adw")
                        if nw < 128:
                            nc.vector.memset(adw[:], 0.0)
                        nc.sync.dma_start(
                            out=adw[0:nw, :],
                            in_=ag_in[w * 128 : w * 128 + nw, COL_AD : COL_AD + 1],
                        )
                        ad_wins[w] = adw

                    # per-tile PE chain: alpha_d per edge
                    adps = ps.tile([P, T_sb], dt.float32, tag="adps")
                    et = sbp.tile([P, T_sb, 1], dt.float32, tag="et")
                    if "no_ad" not in ABLATE:
                        for t in range(T_sb):
                            w = _tile_window(sb, TA, TB, t)
                            bcp = ps.tile([P, P], dt.float32, tag="bcp")
                            nc.tensor.matmul(
                                out=bcp[:],
                                lhsT=ones_row[:],
                                rhs=drow[0:1, t * P : (t + 1) * P],
                                start=True,
                                stop=True,
                            )
                            qt = sbp.tile([P, P], dt.float32, tag="qt", bufs=4)
                            nc.vector.tensor_scalar(
                                out=qt[:], in0=bcp[:], scalar1=iotaP[:, 0:1], scalar2=None,
                                op0=mybir.AluOpType.is_equal,
                            )
                            nc.tensor.matmul(
                                out=adps[:, t : t + 1],
                                lhsT=qt[:],
                                rhs=ad_wins[w][:],
                                start=True,
                                stop=True,
                            )
                        # e = alpha_s + alpha_d ; w = exp(max(e, 0.2e))
                        nc.vector.tensor_tensor(
                            out=et[:],
                            in0=gath[:, :, COL_AS : COL_AS + 1],
                            in1=adps[:, :, None],
                            op=mybir.AluOpType.add,
                        )
                    else:
                        nc.vector.tensor_copy(
                            out=et[:], in_=gath[:, :, COL_AS : COL_AS + 1],
                        )
                    e2 = sbp.tile([P, T_sb, 1], dt.float32, tag="e2")
                    nc.vector.tensor_scalar(
                        out=e2[:], in0=et[:], scalar1=0.2, scalar2=None,
                        op0=mybir.AluOpType.mult,
                    )
                    nc.vector.tensor_tensor(
                        out=e2[:], in0=et[:], in1=e2[:], op=mybir.AluOpType.max,
                    )
                    wt = sbp.tile([P, T_sb, 1], dt.float32, tag="wt")
                    nc.scalar.activation(
                        out=wt[:], in_=e2[:], func=mybir.ActivationFunctionType.Exp,
                    )

                    msg = sbp.tile([P, T_sb, F_OUT + 1], dt.float32, tag="msg")
                    nc.vector.tensor_tensor(
                        out=msg[:, :, 0:F_OUT],
                        in0=gath[:, :, 0:F_OUT],
                        in1=wt[:].to_broadcast([P, T_sb, F_OUT]),
                        op=mybir.AluOpType.mult,
                    )
                    nc.vector.tensor_copy(out=msg[:, :, F_OUT : F_OUT + 1], in_=wt[:])

                    # segment-sum matmuls
                    win_ps = {
                        w: winps.tile([P, F_OUT + 1], dt.float32, tag="win", name=f"win_l{layer}_w{w}")
                        for w in sb
                    }
                    first_seen = {w: True for w in sb}
                    remaining = {w: int(TA[w] + TB[w]) for w in sb}
                    for t in range(T_sb):
                        w = _tile_window(sb, TA, TB, t)
                        if "no_seg" in ABLATE and not (first_seen[w] or remaining[w] == 1):
                            remaining[w] -= 1
                            continue
                        q = sbp.tile([P, P], dt.float32, tag="q", bufs=4)
                        nc.vector.tensor_scalar(
                            out=q[:], in0=iota_row[:], scalar1=dcol[:, t : t + 1], scalar2=None,
                            op0=mybir.AluOpType.is_equal,
                        )
                        nc.tensor.matmul(
                            out=win_ps[w][:],
                            lhsT=q[:],
                            rhs=msg[:, t, :],
                            start=first_seen[w],
                            stop=(remaining[w] == 1),
                        )
                        first_seen[w] = False
                        remaining[w] -= 1

                    # window epilogues
                    for w in sb:
                        pw = win_ps[w]
                        rec = sbp.tile([P, 1], dt.float32, tag="rec")
                        nc.vector.reciprocal(out=rec[:], in_=pw[:, F_OUT : F_OUT + 1])
                        rt = sbp.tile([P, F_OUT], dt.float32, tag="rt")
                        nc.vector.tensor_scalar(
                            out=rt[:], in0=pw[:, 0:F_OUT], scalar1=rec[:, 0:1], scalar2=None,
                            op0=mybir.AluOpType.mult,
                        )
                        nc.vector.tensor_tensor(
                            out=rt[:], in0=rt[:], in1=bb[:], op=mybir.AluOpType.add,
                        )
                        if layer == 1:
                            nc.scalar.activation(
                                out=r1_sb[:, w, :], in_=rt[:],
                                func=mybir.ActivationFunctionType.Relu,
                            )
                        else:
                            nw = min(128, NPC - w * 128)
                            nc.sync.dma_start(
                                out=out_p[w * 128 : w * 128 + nw, :], in_=rt[0:nw, :],
                            )

                    colA += TA_sb * 8
                    colB += TB_sb * 8
                    g0 += T_sb

            with tc.tile_pool(name="psA1", bufs=2, space="PSUM") as psA:
                w1a = mk_wa(psA, W1, A1, F_IN)
                w2a = mk_wa(psA, W2, A2, F_OUT)
                b1b = mk_bcast(psA, b1r)
                b2b = mk_bcast(psA, b2r)
            rep = int(os.environ.get("GAT_REPEAT", "1"))
            for r in range(rep):
                with tc.tile_pool(name=f"psA1_{r}", bufs=2, space="PSUM") as psA:
                    build_table(1, psA, w1a)
                with tc.tile_pool(name=f"psE1_{r}", bufs=2, space="PSUM") as psE:
                    with tc.tile_pool(name=f"winps1_{r}", bufs=2, space="PSUM") as winps:
                        edge_phase(1, psE, winps, b1b)
                with tc.tile_pool(name=f"psA2_{r}", bufs=2, space="PSUM") as psA:
                    build_table(2, psA, w2a)
                with tc.tile_pool(name=f"psE2_{r}", bufs=2, space="PSUM") as psE:
                    with tc.tile_pool(name=f"winps2_{r}", bufs=2, space="PSUM") as winps:
                        edge_phase(2, psE, winps, b2b)

    nc.compile()
    return nc


def _tile_window(sb, TA, TB, t):
    """Map tile index t within a super-batch to its window id."""
    # layout: [A(w) for w in sb] then [B(w) for w in sb]
    off = 0
    for w in sb:
        if t < off + int(TA[w]):
            return w
        off += int(TA[w])
    for w in sb:
        if t < off + int(TB[w]):
            return w
        off += int(TB[w])
    raise AssertionError("tile out of range")


_CACHE = {}


def kernel(**inputs) -> np.ndarray:
    x = np.asarray(inputs["x"], np.float32)
    edge_index = np.asarray(inputs["edge_index"])
    W1 = np.asarray(inputs["W1"], np.float32)
    a1s = np.asarray(inputs["a1_src"], np.float32)
    a1d = np.asarray(inputs["a1_dst"], np.float32)
    b1 = np.asarray(inputs["b1"], np.float32)
    W2 = np.asarray(inputs["W2"], np.float32)
    a2s = np.asarray(inputs["a2_src"], np.float32)
    a2d = np.asarray(inputs["a2_dst"], np.float32)
    b2 = np.asarray(inputs["b2"], np.float32)

    per_core, TA, TB, sbs, NT = _prep_edges(edge_index)

    nc = _build_program(
        TA, TB, sbs, NT,
        per_core[0]["idxA"].shape[1],
        per_core[0]["idxB"].shape[1],
    )

    in_maps = []
    for k in range(C):
        xs = np.zeros((WPC * 128, F_IN), np.float32)
        xs[0:NPC] = x[k * NPC : (k + 1) * NPC]
        in_maps.append(
            {
                "x_shard": xs,
                "W1T": np.ascontiguousarray(W1.T),
                "W1": np.ascontiguousarray(W1),
                "A1": np.ascontiguousarray(np.stack([a1s, a1d], axis=1)),
                "b1": b1[None, :].copy(),
                "W2T": np.ascontiguousarray(W2.T),
                "W2": np.ascontiguousarray(W2),
                "A2": np.ascontiguousarray(np.stack([a2s, a2d], axis=1)),
                "b2": b2[None, :].copy(),
                "idxA": per_core[k]["idxA"],
                "idxB": per_core[k]["idxB"],
                "dcol": per_core[k]["dcol"],
                "drow": per_core[k]["drow"],
            }
        )

    res = run_bass_kernel_spmd(nc, in_maps, core_ids=list(range(C)))
    out = np.concatenate([res.results[k]["out"] for k in range(C)], axis=0)
    return out.astype(np.float32)
